# revision 17
# baseline (speedup 1.0000x reference)
"""Trainium2 Bass kernel for CustomBSplineLayer.

Computes out[b,o] = sum_{i,g} spline(x)[b,i,g] * coef[o,i,g] where
spline is an order-3 (cubic) B-spline basis on uniform knots applied to
tanh(x).

Math used here (validated against the reference recursion):
  u = 3.5*tanh(x) + 3.5           in (0, 7)
  basis_g(u) = M4(u - g)          cardinal cubic B-spline, g = 0..7
  M4(s) = (relu(2-|s-2|)^3 - 4*relu(1-|s-2|)^3) / 6
Plane g=7 is identically zero because its support starts at u=7 ==
tanh upper bound, so only 7 of 8 planes contribute (K = 7*1024 = 7168).

Per-core layout (data-parallel over batch, 8 cores x 512 rows):
  - host pre-transposes x so that tiles arrive as [i partitions, b cols]
  - basis planes computed in [i, b] layout feed the PE directly as the
    stationary (lhsT) operand; coef (host-rearranged to [g, i, o]) is the
    moving operand; out accumulates in PSUM as [b, o] across all 56
    k-tiles, then is copied out once.
  - matmul runs in float32r (tf32) which streams at 1 col/cycle for
    free-dim >= 256 (fp32 is 4 cycles/col).
"""

import sys

sys.path.insert(0, "/opt/trn_rl_repo")

import numpy as np
from contextlib import ExitStack

import concourse.bass as bass
import concourse.tile as tile
from concourse import bacc, mybir
from concourse.bass_utils import run_bass_kernel_spmd

F32 = mybir.dt.float32
F32R = mybir.dt.float32r
I32 = mybir.dt.int32
AF = mybir.ActivationFunctionType
OP = mybir.AluOpType

B, I, O = 4096, 1024, 1024
G = 7                    # active basis planes (plane 7 == 0)
NCORES = 8
BC = B // NCORES         # 512 batch rows per core
IT = I // 128            # 8 i-tiles
KT = IT * G              # 56 k-tiles of 128
WID = G * BC             # 3584: wide free-dim (7 planes x 512 b)

C6 = float(6.0 ** (-1.0 / 3.0))          # folds the 1/6 into p
C46 = float((4.0 / 6.0) ** (1.0 / 3.0))  # folds the 4/6 into q
KQ = float(C46 / C6)                     # q = relu(KQ*p - C46)

# mm dtype: F32R (tf32, fast) or F32 (exact, 4x slower PE)
MM_DT = F32R

LAST_RESULT = None  # BassKernelResults of the most recent run (for test.py)

_cache = {}


def _tf32_round(a: np.ndarray) -> np.ndarray:
    """Round fp32 to tf32 (10-bit mantissa), round-to-nearest-even."""
    bits = np.ascontiguousarray(a, dtype=np.float32).view(np.uint32).copy()
    lsb = (bits >> np.uint32(13)) & np.uint32(1)
    bits += np.uint32(0xFFF) + lsb
    bits &= np.uint32(0xFFFFE000)
    return bits.view(np.float32)


def _build_nc(repeats: int = 1):
    nc = bacc.Bacc("TRN2", target_bir_lowering=False, debug=False)
    xT = nc.dram_tensor("xT", [I, BC], F32, kind="ExternalInput").ap()
    coefT = nc.dram_tensor("coefT", [G, I, O], MM_DT, kind="ExternalInput").ap()
    y = nc.dram_tensor("y", [BC, O], F32, kind="ExternalOutput").ap()

    with tile.TileContext(nc) as tc, ExitStack() as ctx:
        xt_pool = ctx.enter_context(tc.tile_pool(name="xt", bufs=2))
        small = ctx.enter_context(tc.tile_pool(name="small", bufs=2))
        wide = ctx.enter_context(tc.tile_pool(name="wide", bufs=1))
        spl_pool = ctx.enter_context(tc.tile_pool(name="spl", bufs=2))
        rhs_pool = ctx.enter_context(tc.tile_pool(name="rhs", bufs=3))
        out_pool = ctx.enter_context(tc.tile_pool(name="ot", bufs=2))
        psum_pool = ctx.enter_context(
            tc.tile_pool(name="psum", bufs=1, space=bass.MemorySpace.PSUM)
        )

        consts = ctx.enter_context(tc.tile_pool(name="consts", bufs=1))
        bias_p = consts.tile([128, 1], F32, tag="bias_p", name="bias_p")
        nc.gpsimd.memset(bias_p[:], 2.0 * C6)
        bias_q = consts.tile([128, 1], F32, tag="bias_q", name="bias_q")
        nc.gpsimd.memset(bias_q[:], -C46)

        # 8 PSUM banks: [m-tile 0..3] x [o-half 0..1], each [128, 512] f32
        psum = [
            [
                psum_pool.tile(
                    [128, 512], F32, tag=f"ps{m}_{h}", name=f"ps{m}_{h}"
                )
                for h in range(2)
            ]
            for m in range(4)
        ]

        for _rep in range(repeats):
            kt = 0
            for it in range(IT):
                xt = xt_pool.tile([128, BC], F32, tag="xt", name=f"xt{_rep}_{it}")
                nc.gpsimd.dma_start(xt[:], xT[it * 128 : (it + 1) * 128, :])

                t = small.tile([128, BC], F32, tag="t", name=f"t{_rep}_{it}")
                nc.scalar.activation(t[:], xt[:], AF.Tanh)

                # w_g = u - (g+2) = 3.5*t + (1.5-g), then one wide sign-bit
                # clear (int AND) turns all 7 planes into a_g = |w_g| at once
                aw = wide.tile([128, WID], F32, tag="a", name=f"aw{_rep}_{it}")
                for g in range(G):
                    nc.vector.tensor_scalar(
                        aw[:, g * BC : (g + 1) * BC],
                        t[:],
                        3.5,
                        float(1.5 - g),
                        OP.mult,
                        OP.add,
                    )
                awi = aw[:].bitcast(I32)
                nc.vector.tensor_scalar(awi, awi, 0x7FFFFFFF, None, OP.bitwise_and)
                pw = wide.tile([128, WID], F32, tag="p", name=f"pw{_rep}_{it}")
                nc.scalar.activation(
                    pw[:], aw[:], AF.Relu, bias=bias_p[:], scale=-C6
                )
                qw = wide.tile([128, WID], F32, tag="q", name=f"qw{_rep}_{it}")
                nc.scalar.activation(
                    qw[:], pw[:], AF.Relu, bias=bias_q[:], scale=KQ
                )
                p2 = wide.tile([128, WID], F32, tag="p2", name=f"p2{_rep}_{it}")
                nc.scalar.activation(p2[:], pw[:], AF.Square)
                q2 = wide.tile([128, WID], F32, tag="q2", name=f"q2{_rep}_{it}")
                nc.scalar.activation(q2[:], qw[:], AF.Square)
                p3 = wide.tile([128, WID], F32, tag="p3", name=f"p3{_rep}_{it}")
                nc.vector.tensor_tensor(p3[:], p2[:], pw[:], OP.mult)
                q3 = wide.tile([128, WID], F32, tag="q3", name=f"q3{_rep}_{it}")
                nc.vector.tensor_tensor(q3[:], q2[:], qw[:], OP.mult)
                # final subtract writes an fp32r-typed tile: the DVE rounds
                # to tf32 on write, as the fp32r matmul requires of producers
                spl = spl_pool.tile(
                    [128, WID], MM_DT, tag="spl", name=f"spl{_rep}_{it}"
                )
                nc.vector.tensor_tensor(spl[:], p3[:], q3[:], OP.subtract)

                for g in range(G):
                    rhs = rhs_pool.tile(
                        [128, O], MM_DT, tag="rhs", name=f"rhs{_rep}_{it}_{g}"
                    )
                    nc.gpsimd.dma_start(
                        rhs[:], coefT[g, it * 128 : (it + 1) * 128, :]
                    )
                    first = kt == 0
                    last = kt == KT - 1
                    for m in range(4):
                        lhsT = spl[:, g * BC + m * 128 : g * BC + (m + 1) * 128]
                        for h in range(2):
                            nc.tensor.matmul(
                                psum[m][h][:],
                                lhsT,
                                rhs[:, h * 512 : (h + 1) * 512],
                                start=first,
                                stop=last,
                            )
                    kt += 1

            for m in range(4):
                ot = out_pool.tile([128, O], F32, tag="ot", name=f"ot{_rep}_{m}")
                for h in range(2):
                    nc.scalar.copy(ot[:, h * 512 : (h + 1) * 512], psum[m][h][:])
                nc.gpsimd.dma_start(y[m * 128 : (m + 1) * 128, :], ot[:])

    nc.compile()
    return nc


def kernel(x: np.ndarray, coef: np.ndarray) -> np.ndarray:
    global LAST_RESULT
    x = np.asarray(x, dtype=np.float32)
    coef = np.asarray(coef, dtype=np.float32)
    assert x.shape == (B, I) and coef.shape == (O, I, 8)

    if "nc" not in _cache:
        _cache["nc"] = _build_nc()
    nc = _cache["nc"]

    xT = np.ascontiguousarray(x.T)  # [I, B]
    coefT = np.ascontiguousarray(coef.transpose(2, 1, 0)[:G])  # [7, I, O]
    if MM_DT == F32R:
        coefT = _tf32_round(coefT)
    in_maps = [
        {
            "xT": np.ascontiguousarray(xT[:, c * BC : (c + 1) * BC]),
            "coefT": coefT,
        }
        for c in range(NCORES)
    ]
    res = run_bass_kernel_spmd(nc, in_maps, list(range(NCORES)))
    LAST_RESULT = res
    out = np.concatenate([res.results[c]["y"] for c in range(NCORES)], axis=0)
    return np.ascontiguousarray(out.astype(np.float32))


if __name__ == "__main__":
    rng = np.random.default_rng(0)
    x = rng.standard_normal((B, I), dtype=np.float32)
    coef = rng.standard_normal((O, I, 8), dtype=np.float32) * 0.1
    out = kernel(x, coef)
    print("out", out.shape, out.dtype, float(np.abs(out).max()))
